# revision 1
# baseline (speedup 1.0000x reference)
"""MinGRU (L=2, B=8, S=2048, D=H=1024) Trainium2 Bass kernel.

Sharding: data-parallel over batch B across the 8 NeuronCores (1 sequence
per core); the (L,H,D) weights are replicated.

Per-core dataflow (all shapes per core):
  inputs (host-preprocessed): xT (D,S) fp16, WzT/WhT (L,D,H) fp16,
  biases as (L,4,128,H/128) fp32 tiles (bz, -bz, bh, bh+0.5).
  layer l:
    k  = Wz_l @ x          -> PSUM (h-part, s-free), 8 accumulating matmuls
    p  = Wh_l @ x          -> PSUM
    z  = sigmoid(k + bz)          (ACT, PSUM->SBUF)
    c  = sigmoid(-(k + bz))       (ACT, scale=-1, bias=-bz)
    sg = sigmoid(p + bh)          (ACT)
    g  = max(p + bh + 0.5, sg)    (DVE scalar_tensor_tensor; exact identity
                                   for MinGRU's piecewise g)
    v  = z * g                    (DVE)
    h[t] = c[t]*h[t-1] + v[t], h0=0.5   (DVE tensor_tensor_scan, fp32 state)
  layer-0 scan emits fp16 directly into the layer-1 rhs buffer (the scan
  layout IS the next layer's matmul rhs layout — no device transposes);
  layer-1 scan emits fp32 chunks DMA'd to DRAM as (H,S); the host
  transposes back to (S,H). The linear-space scan is numerically safe:
  all terms positive, h bounded in [~1e-3, ~4].
"""

import os
import sys

for _p in (
    "/root/.axon_site",
    "/root/.axon_site/_ro/trn_rl_repo",
    "/root/.axon_site/_ro/pypackages",
    "/opt/trn_rl_repo",
    "/opt/pypackages",
):
    if os.path.isdir(_p) and _p not in sys.path:
        sys.path.append(_p)

from contextlib import ExitStack

import numpy as np

import concourse.bacc as bacc
import concourse.bass as bass
import concourse.tile as tile
from concourse import mybir

L, B, S, D, H = 2, 8, 2048, 1024, 1024
P = 128
DT = D // P          # 8 contraction tiles
HT = H // P          # 8 output-channel tiles
SB = 512             # time-block (one PSUM bank of fp32)
NSB = S // SB        # 4

F16 = mybir.dt.float16
F32 = mybir.dt.float32
AF = mybir.ActivationFunctionType
OP = mybir.AluOpType

LAST_EXEC_NS = None

_BUILT = None


def _build(reps=1, mm_only=False, sb=SB, k_first=False):
    global SB, NSB
    SB, NSB = sb, S // sb
    nc = bacc.Bacc("TRN2", target_bir_lowering=False, debug=False)

    xT = nc.dram_tensor("xT", (D, S), F16, kind="ExternalInput")
    wzT = nc.dram_tensor("wzT", (L, D, H), F16, kind="ExternalInput")
    whT = nc.dram_tensor("whT", (L, D, H), F16, kind="ExternalInput")
    # biases pre-tiled on host: [l, f, p, ht] = bias_f[l, ht*128 + p]
    # f in (bz, -bz, bh, bh+0.5)
    bias_d = nc.dram_tensor("biases", (L, 4, P, HT), F32, kind="ExternalInput")
    outT = nc.dram_tensor("outT", (H, S), F32, kind="ExternalOutput")

    xT_r = xT.rearrange("(dt p) s -> p dt s", p=P)

    with tile.TileContext(nc) as tc, ExitStack() as ctx:
        persist = ctx.enter_context(tc.tile_pool(name="persist", bufs=1))
        cvpool = ctx.enter_context(tc.tile_pool(name="cv", bufs=3))
        zpool = ctx.enter_context(tc.tile_pool(name="zs", bufs=3))
        # layer-1 output chunks: chain distance between same-ht chunks is
        # HT units (x sub-chunks) in sb-major order; keep enough slots live
        ochunk_pool = ctx.enter_context(tc.tile_pool(name="ochunk", bufs=2 * HT + 2))
        pk_pool = ctx.enter_context(tc.tile_pool(name="pk", bufs=2, space="PSUM"))
        pp_pool = ctx.enter_context(tc.tile_pool(name="pp", bufs=2, space="PSUM"))
        warm_pool = ctx.enter_context(tc.tile_pool(name="warm", bufs=1, space="PSUM"))

        # ---- persistent SBUF state ----
        x_sb = persist.tile([P, DT, S], F16)       # layer-0 input (xT)
        h1_sb = persist.tile([P, HT, S], F16)      # layer-0 output = layer-1 rhs
        w_sb = {}
        for l in range(L):
            for nm, dram in (("wz", wzT), ("wh", whT)):
                w_sb[(nm, l)] = persist.tile([P, DT, H], F16, name=f"{nm}{l}_sb")
        bias_tiles = [
            persist.tile([P, 4, HT], F32, name=f"bias{l}_sb") for l in range(L)
        ]
        bias_sb = {}
        for l in range(L):
            for fi, nm in enumerate(("bz", "bzn", "bh", "bh05")):
                bias_sb[(nm, l)] = bias_tiles[l][:, fi]

        def load_w(nm, l, h0, h1):
            # one 3D-AP DMA per slice: per-partition DT chunks of (h1-h0)
            src = {"wz": wzT, "wh": whT}[nm][l].rearrange("(dt p) h -> p dt h", p=P)
            nc.sync.dma_start(out=w_sb[(nm, l)][:, :, h0:h1], in_=src[:, :, h0:h1])

        def load_x(sb):
            nc.sync.dma_start(
                out=x_sb[:, :, sb * SB : (sb + 1) * SB],
                in_=xT_r[:, :, sb * SB : (sb + 1) * SB],
            )

        # PE warmup: dummy matmuls on a zeroed tile run during the DMA
        # lead-in so the HAM clock gate reaches 2.4 GHz before real work.
        # 41 x 213ns ~= 8.7us, sized so PE does not outpace the input DMA
        # stream (fewer warmups create a mid-stream stall that also drops
        # the clock ramp).
        warm = persist.tile([P, SB], F16, name="warm")
        warm_ps = warm_pool.tile([P, SB], F32, name="warm_ps")
        nc.vector.memset(warm, 0.0)
        for _ in range(41):
            nc.tensor.matmul(warm_ps, warm[:, :P], warm, start=True, stop=True)

        # DMA emission in first-consumption order, minimizing DMA count on
        # the critical path (per-DMA queue overhead is significant).
        load_w("wz", 0, 0, P)        # 0.25 MB — first unit's k weights
        load_w("wh", 0, 0, P)        # 0.25 MB — first unit's p weights
        load_x(0)                    # 1 MB
        load_w("wz", 0, P, H // 2)
        load_w("wh", 0, P, H // 2)
        load_w("wz", 0, H // 2, H)
        load_w("wh", 0, H // 2, H)
        nc.sync.dma_start(
            out=bias_tiles[0], in_=bias_d[0].rearrange("f p ht -> p f ht")
        )
        for sb in range(1, NSB):
            load_x(sb)
        nc.sync.dma_start(
            out=bias_tiles[1], in_=bias_d[1].rearrange("f p ht -> p f ht")
        )
        for half in range(2):
            load_w("wz", 1, half * (H // 2), (half + 1) * (H // 2))
            load_w("wh", 1, half * (H // 2), (half + 1) * (H // 2))

        def layer(l, rhs_sb, out_writer, split_last=False):
            """rhs_sb: [P, DT, S] f16 input; out_writer(ht, col0, w, c, v)."""
            wz = w_sb[("wz", l)]
            wh = w_sb[("wh", l)]
            bz_t = bias_sb[("bz", l)]
            bzn_t = bias_sb[("bzn", l)]
            bh_t = bias_sb[("bh", l)]
            bh05_t = bias_sb[("bh05", l)]
            for sb in range(NSB):
                s0, s1 = sb * SB, (sb + 1) * SB
                for ht in range(HT):
                    h0, h1 = ht * P, (ht + 1) * P
                    pk = pk_pool.tile([P, SB], F32, name="pk")
                    pp = pp_pool.tile([P, SB], F32, name="pp")
                    # p-group first: sg/g depend only on p, so they overlap
                    # the k-group's matmuls — shortens each unit's epilogue
                    # and the kernel tail
                    groups = [(pp, wh), (pk, wz)]
                    if k_first:
                        groups = [(pk, wz), (pp, wh)]
                    for ps, wmat in groups:
                        for dt_i in range(DT):
                            nc.tensor.matmul(
                                ps,
                                wmat[:, dt_i, h0:h1],
                                rhs_sb[:, dt_i, s0:s1],
                                start=(dt_i == 0),
                                stop=(dt_i == DT - 1),
                            )
                    if mm_only:
                        continue
                    # shorten the kernel tail: the very last unit's epilogue
                    # runs after the last matmul, so process it in two halves
                    sub = 2 if (split_last and sb == NSB - 1 and ht == HT - 1) else 1
                    w = SB // sub
                    for si in range(sub):
                        c0, c1 = si * w, (si + 1) * w
                        z = zpool.tile([P, SB], F32, name="z")[:, :w]
                        sg = zpool.tile([P, SB], F32, name="sg")[:, :w]
                        g = zpool.tile([P, SB], F32, name="g")[:, :w]
                        c = cvpool.tile([P, SB], F32, name="c")[:, :w]
                        v = cvpool.tile([P, SB], F32, name="v")[:, :w]
                        # ACT order matches the DVE dependency chain: g needs
                        # sg first; the scan needs c last — shortens the
                        # ACT->DVE critical path of each unit (and the tail)
                        nc.scalar.activation(
                            sg, pp[:, c0:c1], AF.Sigmoid,
                            bias=bh_t[:, ht : ht + 1], scale=1.0,
                        )
                        nc.scalar.activation(
                            z, pk[:, c0:c1], AF.Sigmoid,
                            bias=bz_t[:, ht : ht + 1], scale=1.0,
                        )
                        nc.scalar.activation(
                            c, pk[:, c0:c1], AF.Sigmoid,
                            bias=bzn_t[:, ht : ht + 1], scale=-1.0,
                        )
                        # g = (p + (bh+0.5)) max sigmoid(p+bh)
                        nc.vector.scalar_tensor_tensor(
                            g, pp[:, c0:c1], bh05_t[:, ht : ht + 1], sg,
                            op0=OP.add, op1=OP.max,
                        )
                        nc.vector.tensor_mul(v, z, g)
                        out_writer(ht, s0 + c0, w, c, v)

        # layer 0: scan into h1_sb (f16), chained across blocks
        def l0_writer(ht, col0, w, c, v):
            dst = h1_sb[:, ht, col0 : col0 + w]
            init = 0.5 if col0 == 0 else h1_sb[:, ht, col0 - 1 : col0]
            nc.vector.tensor_tensor_scan(dst, c, v, init, op0=OP.mult, op1=OP.add)

        # layer 1: scan into fp32 chunks, DMA out per chunk
        prev_chunk = {}

        def l1_writer(ht, col0, w, c, v):
            oc = ochunk_pool.tile([P, SB], F32, name="oc")[:, :w]
            if col0 == 0:
                init = 0.5
            else:
                pt, pw = prev_chunk[ht]
                init = pt[:, pw - 1 : pw]
            nc.vector.tensor_tensor_scan(oc, c, v, init, op0=OP.mult, op1=OP.add)
            prev_chunk[ht] = (oc, w)
            nc.sync.dma_start(out=outT[ht * P : (ht + 1) * P, col0 : col0 + w], in_=oc)

        def body():
            layer(0, x_sb, l0_writer)
            layer(1, x_sb if mm_only else h1_sb, l1_writer, split_last=True)

        if reps == 1:
            body()
        else:
            # timing-only: run the body `reps` times in a hardware loop so
            # one dispatch amortizes the host->terminal RPC floor
            with tc.For_i(0, reps, 1, hint_engines=tuple(nc.engines)):
                body()

    nc.finalize()
    return nc


class _Runner:
    """Compile the bass module once into a jitted shard_map over 8 cores."""

    def __init__(self, reps=1, mm_only=False, sb=512, k_first=False):
        import jax
        from jax.experimental.shard_map import shard_map
        from jax.sharding import Mesh, NamedSharding, PartitionSpec

        from concourse import bass2jax, mybir as _mybir

        self.jax = jax
        nc = _build(reps, mm_only=mm_only, sb=sb, k_first=k_first)
        self.nc = nc
        bass2jax.install_neuronx_cc_hook()

        partition_name = (
            nc.partition_id_tensor.name if nc.partition_id_tensor else None
        )
        in_names, out_names, out_avals, zero_shapes = [], [], [], []
        for alloc in nc.m.functions[0].allocations:
            if not isinstance(_mybir.MemoryLocationSet, type) or not isinstance(
                alloc, _mybir.MemoryLocationSet
            ):
                continue
            name = alloc.memorylocations[0].name
            if alloc.kind == "ExternalInput":
                if name != partition_name:
                    in_names.append(name)
            elif alloc.kind == "ExternalOutput":
                shape = tuple(alloc.tensor_shape)
                dtype = _mybir.dt.np(alloc.dtype)
                out_names.append(name)
                out_avals.append(jax.core.ShapedArray(shape, dtype))
                zero_shapes.append((shape, dtype))
        self.in_names = list(in_names)
        self.out_names = out_names
        self.zero_shapes = zero_shapes
        n_params = len(in_names)
        n_outs = len(out_names)
        all_in_names = in_names + out_names
        if partition_name is not None:
            all_in_names.append(partition_name)
        donate = tuple(range(n_params, n_params + n_outs))

        def _body(*args):
            operands = list(args)
            if partition_name is not None:
                operands.append(bass2jax.partition_id_tensor())
            outs = bass2jax._bass_exec_p.bind(
                *operands,
                out_avals=tuple(out_avals),
                in_names=tuple(all_in_names),
                out_names=tuple(out_names),
                lowering_input_output_aliases=(),
                sim_require_finite=True,
                sim_require_nnan=True,
                nc=nc,
            )
            return tuple(outs)

        self._base_body = _body
        devices = jax.devices()[:B]
        assert len(devices) == B
        self.mesh = Mesh(np.asarray(devices), ("core",))
        self.sharding = NamedSharding(self.mesh, PartitionSpec("core"))
        in_specs = (PartitionSpec("core"),) * (n_params + n_outs)
        out_specs = (PartitionSpec("core"),) * n_outs
        _mapped = shard_map(
            _body,
            mesh=self.mesh,
            in_specs=in_specs,
            out_specs=out_specs,
            check_rep=False,
        )
        self.fn = jax.jit(_mapped, donate_argnums=donate, keep_unused=True)
        self.fn_nodonate = jax.jit(_mapped, keep_unused=True)

    def _concat_inputs(self, in_maps):
        return [
            np.concatenate([np.asarray(m[name]) for m in in_maps], axis=0)
            for name in self.in_names
        ]

    def _zeros(self):
        return [
            np.zeros((B * s[0], *s[1:]), dt) for (s, dt) in self.zero_shapes
        ]

    def run(self, in_maps):
        out_arrs = self.fn(*self._concat_inputs(in_maps), *self._zeros())
        return [
            {
                name: np.asarray(out_arrs[i]).reshape(B, -1, *out_arrs[i].shape[1:])[c]
                for i, name in enumerate(self.out_names)
            }
            for c in range(B)
        ]

    def bench_loop(self, in_maps, iters=16, inner=4):
        """Min wall time of `inner` back-to-back non-donating executions."""
        import time as _time

        jax = self.jax
        dev_in = [
            jax.device_put(a, self.sharding) for a in self._concat_inputs(in_maps)
        ]
        dev_z = [jax.device_put(z, self.sharding) for z in self._zeros()]
        out = self.fn_nodonate(*dev_in, *dev_z)
        jax.block_until_ready(out)
        best = float("inf")
        for _ in range(iters):
            t0 = _time.perf_counter()
            for _ in range(inner):
                out = self.fn_nodonate(*dev_in, *dev_z)
            jax.block_until_ready(out)
            best = min(best, (_time.perf_counter() - t0) / inner)
        return best * 1e9

    def bench(self, in_maps, iters=8):
        """Return (est_ns_per_iter, results_of_last)."""
        import time as _time

        jax = self.jax
        dev_in = [
            jax.device_put(a, self.sharding) for a in self._concat_inputs(in_maps)
        ]
        zero_sets = [
            [jax.device_put(z, self.sharding) for z in self._zeros()]
            for _ in range(iters + 1)
        ]
        out = self.fn(*dev_in, *zero_sets[0])  # warmup
        jax.block_until_ready(out)
        t0 = _time.perf_counter()
        for i in range(iters):
            out = self.fn(*dev_in, *zero_sets[i + 1])
        jax.block_until_ready(out)
        t1 = _time.perf_counter()
        est_ns = (t1 - t0) / iters * 1e9
        results = [
            {
                name: np.asarray(out[i]).reshape(B, -1, *out[i].shape[1:])[c]
                for i, name in enumerate(self.out_names)
            }
            for c in range(B)
        ]
        return est_ns, results


_RUNNER = None
_LAST_IN_MAPS = None


def _get_runner():
    global _RUNNER
    if _RUNNER is None:
        _RUNNER = _Runner()
    return _RUNNER


def _preprocess(x, Wz, bz, Wh, bh):
    x = np.asarray(x, dtype=np.float32)
    Wz = np.asarray(Wz, dtype=np.float32)
    bz = np.asarray(bz, dtype=np.float32)
    Wh = np.asarray(Wh, dtype=np.float32)
    bh = np.asarray(bh, dtype=np.float32)

    bf = np.float16
    xT = np.ascontiguousarray(x.transpose(0, 2, 1)).astype(bf)        # (B, D, S)
    wzT = np.ascontiguousarray(Wz.transpose(0, 2, 1)).astype(bf)      # (L, D, H)
    whT = np.ascontiguousarray(Wh.transpose(0, 2, 1)).astype(bf)

    def tile_bias(b):  # (L, H) -> (L, P, HT) with [l, p, ht] = b[l, ht*P + p]
        return np.ascontiguousarray(
            b.reshape(L, HT, P).transpose(0, 2, 1)
        ).astype(np.float32)

    biases = np.ascontiguousarray(
        np.stack(
            [tile_bias(bz), tile_bias(-bz), tile_bias(bh), tile_bias(bh + 0.5)],
            axis=1,
        )
    )  # (L, 4, P, HT)

    return [
        {"xT": xT[b], "wzT": wzT, "whT": whT, "biases": biases}
        for b in range(B)
    ]


def kernel(x, Wz, bz, Wh, bh, _bench_iters=0):
    global LAST_EXEC_NS, _LAST_IN_MAPS
    runner = _get_runner()
    in_maps = _preprocess(x, Wz, bz, Wh, bh)
    _LAST_IN_MAPS = in_maps
    if _bench_iters:
        LAST_EXEC_NS, results = runner.bench(in_maps, iters=_bench_iters)
    else:
        results = runner.run(in_maps)
    out = np.stack([results[b]["outT"].T for b in range(B)], axis=0)
    return np.ascontiguousarray(out.astype(np.float32))

